# revision 1
# baseline (speedup 1.0000x reference)
"""Single-head attention (B=4, N=2048, D=1024, fp32 I/O) on 8 TRN2 NeuronCores.

Sharding: data-parallel over (batch, sequence-half): core i handles batch i//2,
query rows (i%2)*1024:(i%2+1)*1024.  No collectives — each core receives the
full 2048 keys of its batch (its own query rows permuted first; attention is
permutation-invariant over keys) and computes k/v projections locally.

Weights are passed host-side as bf16 W^T (a storage-layout choice, as a bf16
serving stack would keep them); x stays fp32 and is cast+transposed on device
(SWDGE cast-DMA then xbar DMA-transpose, batched by kind since
DMACopy<->DMATranspose transitions serialize on the xbar).

Per core:
  qT[d,n] = Wq @ x^T + bq   (TensorE, bf16 in / f32 psum, bias on ACT eviction)
  kT[d,m], v[m,d] likewise  (v bias added on DVE eviction)
  per 128-query block nb: S = q@k^T into 4 single-bank psum tiles,
  P = exp(S/32) per bank (ACT), P^T via TensorE transposes into the same
  psum banks, row-sums + P^T copies on DVE, out = P^T.T @ v * (1/rowsum).
  Out-blocks run one block behind S-blocks so PE never stalls on the
  softmax epilogue.
"""

import numpy as np
import ml_dtypes

import concourse.bass as bass
import concourse.bacc as bacc
import concourse.mybir as mybir
import concourse.tile as tile
from concourse.bass_utils import run_bass_kernel_spmd
from concourse.masks import make_identity

B, N, D = 4, 2048, 1024
P = 128
NCORES = 8
HALF = N // 2              # 1024 query rows per core
SCALE = float(D) ** -0.5   # 1/32

F32 = mybir.dt.float32
BF16 = mybir.dt.bfloat16


def build_nc():
    nc = bacc.Bacc("TRN2", target_bir_lowering=False)

    x_h = nc.declare_dram_parameter("x", [N, D], F32, isOutput=False)
    wqt_h = nc.declare_dram_parameter("wqt", [D, D], BF16, isOutput=False)  # Wq^T
    wkt_h = nc.declare_dram_parameter("wkt", [D, D], BF16, isOutput=False)
    wvt_h = nc.declare_dram_parameter("wvt", [D, D], BF16, isOutput=False)
    bqt_h = nc.declare_dram_parameter("bqt", [P, 8], F32, isOutput=False)
    bkt_h = nc.declare_dram_parameter("bkt", [P, 8], F32, isOutput=False)
    bv_h = nc.declare_dram_parameter("bv", [1, D], BF16, isOutput=False)
    out_h = nc.declare_dram_parameter("out", [HALF, D], F32, isOutput=True)

    Exp = mybir.ActivationFunctionType.Exp
    Ident = mybir.ActivationFunctionType.Identity
    AX = mybir.AxisListType.X
    ADD = mybir.AluOpType.add

    with (
        tile.TileContext(nc) as tc,
        tc.tile_pool(name="singles", bufs=1) as singles,
        tc.tile_pool(name="stage", bufs=10) as stage,
        tc.tile_pool(name="pwork", bufs=2) as pwork,
        tc.tile_pool(name="psS", bufs=1, space="PSUM") as psS,
        tc.tile_pool(name="psB", bufs=2, space="PSUM") as psB,
        tc.tile_pool(name="psO", bufs=2, space="PSUM") as psO,
    ):
        # ---- persistent SBUF tensors ----
        xT = singles.tile([P, 16, 8, P], BF16)   # x^T: [p, rb, j, nn]
        wqT = singles.tile([P, 8, D], BF16)      # W^T: [p, cc, d]
        wkT = singles.tile([P, 8, D], BF16)
        wvT = singles.tile([P, 8, D], BF16)
        qT = singles.tile([P, 8, HALF], BF16)    # [p, dc, n]
        kT = singles.tile([P, 8, N], BF16)       # [p, dc, m]
        vv = singles.tile([P, 16, D], BF16)      # [p, mc, d]
        vb = singles.tile([P, D], BF16)
        bqt = singles.tile([P, 8], F32)
        bkt = singles.tile([P, 8], F32)
        ident = singles.tile([P, P], BF16)
        make_identity(nc, ident[:])

        # ---- stage A ----
        nc.sync.dma_start(out=bqt[:], in_=bqt_h[:, :])
        nc.sync.dma_start(out=bkt[:], in_=bkt_h[:, :])
        bv_ap = bv_h[:, :]
        bv_bcast = bass.AP(
            tensor=bv_ap.tensor,
            offset=bv_ap.offset,
            ap=[[0, P]] + list(bv_ap.ap[1:]),
        )
        nc.gpsimd.dma_start(out=vb[:], in_=bv_bcast)

        def w_load(wt_h, wt):
            # bf16 W^T from DRAM on the sync HWDGE queue: FIFO-sequenced
            # between transpose groups so these copies never overlap an
            # xbar-mode flip (copies in flight poison every flip globally).
            nc.sync.dma_start(
                out=wt[:],
                in_=wt_h[:, :].rearrange("(cc p) d -> p cc d", p=P),
            )

        def x_casts(rbs):
            bufs = []
            for rb in rbs:
                buf = stage.tile([P, D], BF16, tag="stg")
                nc.gpsimd.dma_start(
                    out=buf[:], in_=x_h[rb * P : (rb + 1) * P, :]
                )
                bufs.append(buf)
            return bufs

        def x_xposes(rbs, bufs):
            for rb, buf in zip(rbs, bufs):
                nc.sync.dma_start_transpose(out=xT[:, rb, :, :], in_=buf[:])

        # Each cast group is emitted just before its own transpose group:
        # a transpose's static wait-set only covers copies scheduled before
        # it, so group 0 flips after just wk + 4 casts (~12us) instead of
        # waiting out the whole cast stream.
        bufs03 = x_casts(range(0, 4))
        w_load(wkt_h, wkT)
        x_xposes(range(0, 4), bufs03)
        w_load(wqt_h, wqT)
        bufs47 = x_casts(range(4, 8))
        x_xposes(range(4, 8), bufs47)
        bufs811 = x_casts(range(8, 12))
        x_xposes(range(8, 12), bufs811)
        w_load(wvt_h, wvT)
        bufs1215 = x_casts(range(12, 16))
        x_xposes(range(12, 16), bufs1215)

        # ---- stage B: projections ----
        if True:
            # kT first (S needs all keys); mq outer so early blocks unblock
            # asap.  q-proj is sandwiched after the first two key quarters:
            # it only needs xT groups 0-1, giving stage A slack to deliver
            # xT8-15 before k mq2/mq3 and v consume them.
            for mq in (0, 1):
                for dc in range(8):
                    ps = psB.tile([P, 512], F32, tag="psb")
                    for cc in range(8):
                        nc.tensor.matmul(
                            ps[:],
                            lhsT=wkT[:, cc, dc * P : (dc + 1) * P],
                            rhs=xT[:, mq * 4 : (mq + 1) * 4, cc, :],
                            start=(cc == 0),
                            stop=(cc == 7),
                        )
                    nc.scalar.activation(
                        out=kT[:, dc, mq * 512 : (mq + 1) * 512],
                        in_=ps[:],
                        func=Ident,
                        bias=bkt[:, dc : dc + 1],
                        scale=1.0,
                    )

            for h2 in range(2):
                for dc in range(8):
                    ps = psB.tile([P, 512], F32, tag="psb")
                    for cc in range(8):
                        nc.tensor.matmul(
                            ps[:],
                            lhsT=wqT[:, cc, dc * P : (dc + 1) * P],
                            rhs=xT[:, h2 * 4 : (h2 + 1) * 4, cc, :],
                            start=(cc == 0),
                            stop=(cc == 7),
                        )
                    nc.scalar.activation(
                        out=qT[:, dc, h2 * 512 : (h2 + 1) * 512],
                        in_=ps[:],
                        func=Ident,
                        bias=bqt[:, dc : dc + 1],
                        scale=1.0,
                    )

            for mq in (2, 3):
                for dc in range(8):
                    ps = psB.tile([P, 512], F32, tag="psb")
                    for cc in range(8):
                        nc.tensor.matmul(
                            ps[:],
                            lhsT=wkT[:, cc, dc * P : (dc + 1) * P],
                            rhs=xT[:, mq * 4 : (mq + 1) * 4, cc, :],
                            start=(cc == 0),
                            stop=(cc == 7),
                        )
                    nc.scalar.activation(
                        out=kT[:, dc, mq * 512 : (mq + 1) * 512],
                        in_=ps[:],
                        func=Ident,
                        bias=bkt[:, dc : dc + 1],
                        scale=1.0,
                    )

            for mc in range(16):
                for dh in range(2):
                    ps = psB.tile([P, 512], F32, tag="psb")
                    for cc in range(8):
                        nc.tensor.matmul(
                            ps[:],
                            lhsT=xT[:, mc, cc, :],
                            rhs=wvT[:, cc, dh * 512 : (dh + 1) * 512],
                            start=(cc == 0),
                            stop=(cc == 7),
                        )
                    nc.vector.tensor_tensor(
                        out=vv[:, mc, dh * 512 : (dh + 1) * 512],
                        in0=ps[:],
                        in1=vb[:, dh * 512 : (dh + 1) * 512],
                        op=ADD,
                    )

        # ---- stage C ----
        if True:

            def emit_out_block(nb, PT, recip):
                po0 = psO.tile([P, 512], F32, tag="po")
                po1 = psO.tile([P, 512], F32, tag="po")
                for mc in range(16):
                    nc.tensor.matmul(
                        po0[:], lhsT=PT[:, mc, :], rhs=vv[:, mc, 0:512],
                        start=(mc == 0), stop=(mc == 15),
                    )
                    nc.tensor.matmul(
                        po1[:], lhsT=PT[:, mc, :], rhs=vv[:, mc, 512:1024],
                        start=(mc == 0), stop=(mc == 15),
                    )
                outsb = pwork.tile([P, D], F32, tag="outsb")
                nc.vector.tensor_scalar_mul(
                    out=outsb[:, 0:512], in0=po0[:], scalar1=recip[:]
                )
                nc.vector.tensor_scalar_mul(
                    out=outsb[:, 512:1024], in0=po1[:], scalar1=recip[:]
                )
                nc.sync.dma_start(out=out_h[nb * P : (nb + 1) * P, :], in_=outsb[:])

            pending = None
            for nb in range(8):
                # S in four single-bank tiles so exps pipeline per-bank
                Sq = []
                for mq in range(4):
                    s = psS.tile([P, 512], F32, tag=f"S{mq}")
                    Sq.append(s)
                    for dc in range(8):
                        nc.tensor.matmul(
                            s[:],
                            lhsT=qT[:, dc, nb * P : (nb + 1) * P],
                            rhs=kT[:, dc, mq * 512 : (mq + 1) * 512],
                            start=(dc == 0),
                            stop=(dc == 7),
                        )

                sums = pwork.tile([P, 4], F32, tag="sums")
                PT = pwork.tile([P, 16, P], BF16, tag="PT")
                for mq in range(4):
                    Ptq = pwork.tile([P, 512], BF16, tag=f"P{mq}")
                    nc.scalar.activation(
                        out=Ptq[:],
                        in_=Sq[mq][:],
                        func=Exp,
                        scale=SCALE,
                    )
                    # P^T staging reuses this quarter's S psum bank
                    ptq = psS.tile([P, 4, P], BF16, tag=f"S{mq}")
                    for j in range(4):
                        nc.tensor.transpose(
                            ptq[:, j, :],
                            Ptq[:, j * P : (j + 1) * P],
                            ident[:],
                        )
                    nc.vector.tensor_copy(
                        out=PT[:, mq * 4 : (mq + 1) * 4, :], in_=ptq[:]
                    )
                    nc.vector.tensor_reduce(
                        out=sums[:, mq : mq + 1], in_=Ptq[:], axis=AX, op=ADD
                    )
                den = pwork.tile([P, 1], F32, tag="den")
                nc.vector.tensor_reduce(out=den[:], in_=sums[:], axis=AX, op=ADD)
                recip = pwork.tile([P, 1], F32, tag="recip")
                nc.vector.reciprocal(recip[:], den[:])

                if pending is not None:
                    emit_out_block(*pending)
                pending = (nb, PT, recip)
            emit_out_block(*pending)

    nc.finalize()
    return nc


def make_in_maps(x, Wq, bq, Wk, bk, Wv, bv):
    x = np.asarray(x, np.float32)
    bf = ml_dtypes.bfloat16
    wqt = np.ascontiguousarray(np.asarray(Wq, np.float32).T).astype(bf)
    wkt = np.ascontiguousarray(np.asarray(Wk, np.float32).T).astype(bf)
    wvt = np.ascontiguousarray(np.asarray(Wv, np.float32).T).astype(bf)
    bqt = np.ascontiguousarray(np.asarray(bq, np.float32).reshape(8, P).T)
    bkt = np.ascontiguousarray(np.asarray(bk, np.float32).reshape(8, P).T)
    bvr = np.ascontiguousarray(np.asarray(bv, np.float32).reshape(1, D)).astype(bf)
    in_maps = []
    for i in range(NCORES):
        b, h = divmod(i, 2)
        xb = x[b]
        xp = np.ascontiguousarray(
            np.concatenate(
                [xb[h * HALF : (h + 1) * HALF], xb[(1 - h) * HALF : (2 - h) * HALF]],
                axis=0,
            )
        )
        in_maps.append(
            {
                "x": xp,
                "wqt": wqt,
                "wkt": wkt,
                "wvt": wvt,
                "bqt": bqt,
                "bkt": bkt,
                "bv": bvr,
            }
        )
    return in_maps


def gather_out(results):
    out = np.empty((B, N, D), np.float32)
    for i in range(NCORES):
        b, h = divmod(i, 2)
        out[b, h * HALF : (h + 1) * HALF] = results[i]["out"]
    return out


def kernel(x, Wq, bq, Wk, bk, Wv, bv):
    nc = build_nc()
    in_maps = make_in_maps(x, Wq, bq, Wk, bk, Wv, bv)
    res = run_bass_kernel_spmd(nc, in_maps, core_ids=list(range(NCORES)))
    return gather_out(res.results)

